# revision 14
# baseline (speedup 1.0000x reference)
"""Trainium2 Bass kernel for nn_Assistance (colors_only path).

Computes, for each of 64x64=4096 patches (21x21 window, stride 2) of a
147x147x3 image: 3 wedge indicators from 5 params (ests), then the
wedge-weighted mean colors -> output (1, 3, 3, 64, 64).

Sharding: 8 cores, each takes 8 consecutive patch rows (hp), i.e. 512
patches. Each core receives its 35 relevant image rows and its ests
shard so the SPMD graph is core-independent. No collectives needed.

Key reformulation (saves ops): with h0, h1 the two sigmoid-ish
indicators and p01 = h0*h1,
  num_k,c for k=0,1,2  =  S_c - M1_c,  M1_c - M2_c,  M2_c
  wsum_k               =  441 - H1,    H1 - H2,      H2
where S_c = sum(I_c), M1_c = sum(h0*I_c), M2_c = sum(p01*I_c),
H1 = sum(h0), H2 = sum(p01). So only 6 fused multiply-accumulate ops
and 5 plain accumulations per patch-group instead of 9+3 + wedges.
"""
import os
import sys

for _p in ("/opt/trn_rl_repo", "/root/.axon_site/_ro/trn_rl_repo"):
    if os.path.isdir(_p) and _p not in sys.path:
        sys.path.insert(0, _p)

import numpy as np

import concourse.bass as bass
import concourse.bacc as bacc
import concourse.tile as tile
from concourse import mybir
from concourse.bass_utils import run_bass_kernel_spmd

F32 = mybir.dt.float32
OP = mybir.AluOpType
ACT = mybir.ActivationFunctionType

PI = float(np.pi)
R = 21
STRIDE = 2
ETA = 0.01
TAU = 0.1
H = W = 147
HP = WP = 64
NPIX = R * R  # 441
NCORES = 8
GROUPS = 4          # 4 groups of 128 patches per core
ROWS_PER_CORE = 35  # image rows needed per core (8 hp rows + halo)


def _fit_sincos_coeffs():
    """Least-squares poly coeffs for -sin(v), -cos(v) on [-pi, pi].

    With v = (a mod 2pi) - pi:  sin(a) = -sin(v), cos(a) = -cos(v),
    so these polys evaluate sin(a), cos(a) directly.
    """
    v = np.linspace(-PI, PI, 20001)
    A = np.stack([v ** (2 * k + 1) for k in range(7)], 1)
    cs = np.linalg.lstsq(A, -np.sin(v), rcond=None)[0]
    Ac = np.stack([v ** (2 * k) for k in range(8)], 1)
    cc = np.linalg.lstsq(Ac, -np.cos(v), rcond=None)[0]
    return [float(x) for x in cs], [float(x) for x in cc]


SIN_C, COS_C = _fit_sincos_coeffs()


def build_nc():
    nc = bacc.Bacc()

    img_ext = nc.declare_dram_parameter("img", [ROWS_PER_CORE * W * 3], F32, isOutput=False)
    ests_ext = nc.declare_dram_parameter("ests", [512 * 5], F32, isOutput=False)
    xg_ext = nc.declare_dram_parameter("xg", [NPIX], F32, isOutput=False)
    yg_ext = nc.declare_dram_parameter("yg", [NPIX], F32, isOutput=False)
    id_ext = nc.declare_dram_parameter("ident", [128, 128], F32, isOutput=False)
    out_ext = nc.declare_dram_parameter("out", [9, 512], F32, isOutput=True)

    def bcast(ext, n):
        a = ext[:]
        return bass.AP(tensor=a.tensor, offset=a.offset, ap=[[0, 128], [1, n]])

    def dram_ap(ext, offset, dims):
        a = ext[:]
        return bass.AP(tensor=a.tensor, offset=a.offset + offset, ap=dims)

    with tile.TileContext(nc) as tc:
        with (
            tc.tile_pool(name="const", bufs=1) as const,
            tc.tile_pool(name="sc", bufs=1) as sc,      # per-patch scalars (live whole kernel)
            tc.tile_pool(name="patch", bufs=2) as patchp,
            tc.tile_pool(name="work", bufs=2) as work,
            tc.tile_pool(name="scr", bufs=2) as scrp,
            tc.tile_pool(name="psum", bufs=1, space="PSUM") as psum,
        ):
            # ---------------- constants ----------------
            X = const.tile([128, NPIX], F32)
            Y = const.tile([128, NPIX], F32)
            nc.sync.dma_start(out=X, in_=bcast(xg_ext, NPIX))
            nc.sync.dma_start(out=Y, in_=bcast(yg_ext, NPIX))
            ident = const.tile([128, 128], F32)
            nc.sync.dma_start(out=ident, in_=id_ext[:, :])

            # ---------------- ests load ----------------
            # E[p, g, q] = ests[g*128 + p, q]
            E = const.tile([128, GROUPS, 5], F32)
            nc.sync.dma_start(
                out=E, in_=dram_ap(ests_ext, 0, [[5, 128], [128 * 5, GROUPS], [1, 5]])
            )

            # ---------------- per-patch scalar prep ([128, 4] tiles) ----------------
            G = GROUPS

            def t4(tag):
                return sc.tile([128, G], F32, name=tag, tag=tag)

            def ts(out, in0, s1, s2=None, op0=OP.mult, op1=OP.add):
                if s2 is None:
                    nc.vector.tensor_scalar(out=out, in0=in0, scalar1=s1, scalar2=None, op0=op0)
                else:
                    nc.vector.tensor_scalar(out=out, in0=in0, scalar1=s1, scalar2=s2, op0=op0, op1=op1)

            def stt(out, in0, s, in1, op0, op1, accum_out=None):
                nc.vector.scalar_tensor_tensor(
                    out=out, in0=in0, scalar=s, in1=in1, op0=op0, op1=op1,
                    **({"accum_out": accum_out} if accum_out is not None else {}),
                )

            def tt(out, a, b, op):
                nc.vector.tensor_tensor(out=out, in0=a, in1=b, op=op)

            # theta_j = mod((e_j + 1)*pi, 2*pi), j = 0..2
            th = []
            for j in range(3):
                ej = E[:, :, j]
                t = t4(f"t{j}")
                ts(t, ej, PI, PI, OP.mult, OP.add)          # (e+1)*pi
                u = t4(f"u{j}")
                ts(u, t, 1.0 / (2 * PI))                     # t / 2pi
                f = t4(f"f{j}a")
                nc.vector.memset(f, -3.0)
                for i, thr in enumerate((-2.0, -1.0, 0.0, 1.0, 2.0, 3.0)):
                    f2 = t4(f"f{j}" + ("a" if i % 2 else "b"))
                    stt(f2, u, thr, f, OP.is_ge, OP.add)     # f += (u >= thr)
                    f = f2
                tj = t4(f"th{j}")
                stt(tj, f, -2 * PI, t, OP.mult, OP.add)      # t - 2pi*floor
                th.append(tj)

            a1 = t4("a1"); a3 = t4("a3"); a2 = t4("a2")
            tmp = t4("tmp")
            tt(tmp, th[0], th[1], OP.min); tt(a1, tmp, th[2], OP.min)
            tmp2 = t4("tmp2")
            tt(tmp2, th[0], th[1], OP.max); tt(a3, tmp2, th[2], OP.max)
            ssum = t4("ssum")
            tt(ssum, th[0], th[1], OP.add)
            ssum2 = t4("ssum2")
            tt(ssum2, ssum, th[2], OP.add)
            s_m1 = t4("s_m1")
            tt(s_m1, ssum2, a1, OP.subtract)
            tt(a2, s_m1, a3, OP.subtract)

            x0 = t4("x0"); y0 = t4("y0")
            ts(x0, E[:, :, 3], 3.0)
            ts(y0, E[:, :, 4], 3.0)

            # a4 = 0.5*(a1+a3) + pi * [mod(0.5*(a1-a3), 2pi) >= pi]
            df = t4("df"); nh = t4("nh")
            ts(nh, a3, -0.5)
            stt(df, a1, 0.5, nh, OP.mult, OP.add)            # 0.5*(a1-a3), in (-pi, 0]
            c0 = t4("c0")
            ts(c0, df, 0.0, None, OP.is_lt)                  # df < 0
            m4 = t4("m4")
            stt(m4, c0, 2 * PI, df, OP.mult, OP.add)         # mod value
            ge = t4("ge")
            ts(ge, m4, PI, None, OP.is_ge)
            gp = t4("gp")
            ts(gp, ge, PI)
            a4h = t4("a4h")
            stt(a4h, a3, 0.5, gp, OP.mult, OP.add)
            a4 = t4("a4")
            stt(a4, a1, 0.5, a4h, OP.mult, OP.add)

            # D13 = a3 - a1 (already in [0, 2pi)); D42 = mod(a2 - a4, 2pi)
            D13 = t4("D13")
            tt(D13, a3, a1, OP.subtract)
            dd = t4("dd")
            tt(dd, a2, a4, OP.subtract)                      # in (-3pi, 2pi)
            ud = t4("ud")
            ts(ud, dd, 1.0 / (2 * PI))
            fd = t4("fda")
            nc.vector.memset(fd, -2.0)
            for i, thr in enumerate((-1.0, 0.0, 1.0)):
                fd2 = t4("fd" + ("a" if i % 2 else "b"))
                stt(fd2, ud, thr, fd, OP.is_ge, OP.add)
                fd = fd2
            D42 = t4("D42")
            stt(D42, fd, -2 * PI, dd, OP.mult, OP.add)

            # sgn = 2*[D < pi] - 1  (and negated)
            sgn13 = t4("sgn13"); nsgn13 = t4("nsgn13")
            sgn42 = t4("sgn42"); nsgn42 = t4("nsgn42")
            cl = t4("cl")
            ts(cl, D13, PI, None, OP.is_lt)
            ts(sgn13, cl, 2.0, -1.0, OP.mult, OP.add)
            ts(nsgn13, sgn13, -1.0)
            cl2 = t4("cl2")
            ts(cl2, D42, PI, None, OP.is_lt)
            ts(sgn42, cl2, 2.0, -1.0, OP.mult, OP.add)
            ts(nsgn42, sgn42, -1.0)

            # gt = tau * (D/pi - 1)^35
            def pow35(src, tag):
                v = t4(tag + "v")
                ts(v, src, 1.0 / PI, -1.0, OP.mult, OP.add)
                v2 = t4(tag + "2"); tt(v2, v, v, OP.mult)
                v3 = t4(tag + "3"); tt(v3, v2, v, OP.mult)
                v4 = t4(tag + "4"); tt(v4, v2, v2, OP.mult)
                v8 = t4(tag + "8"); tt(v8, v4, v4, OP.mult)
                v16 = t4(tag + "16"); tt(v16, v8, v8, OP.mult)
                v32 = t4(tag + "32"); tt(v32, v16, v16, OP.mult)
                v35 = t4(tag + "35"); tt(v35, v32, v3, OP.mult)
                gt = t4(tag + "gt")
                ts(gt, v35, TAU)
                return gt

            gt13 = pow35(D13, "g13")
            gt42 = pow35(D42, "g42")

            # ---- sin/cos of a1..a4 on a stacked [128, 16] tile ----
            A = sc.tile([128, 4, G], F32, name="angles", tag="angles")
            for i, a in enumerate((a1, a2, a3, a4)):
                nc.vector.tensor_copy(out=A[:, i, :], in_=a)
            AF = A[:, :, :].rearrange("p a g -> p (a g)")

            def t16(tag):
                return sc.tile([128, 4 * G], F32, name=tag, tag=tag)

            uw = t16("uw")
            nc.vector.tensor_scalar(out=uw, in0=AF, scalar1=2 * PI, scalar2=None, op0=OP.is_ge)
            ar = t16("ar")
            nc.vector.scalar_tensor_tensor(out=ar, in0=uw, scalar=-2 * PI, in1=AF, op0=OP.mult, op1=OP.add)
            vv = t16("vv")
            nc.vector.tensor_scalar(out=vv, in0=ar, scalar1=-PI, scalar2=None, op0=OP.add)
            v2 = t16("v2")
            tt(v2, vv, vv, OP.mult)
            # sin(a): odd poly in v (coeffs of -sin(v))
            ps = t16("psa")
            ts(ps, v2, SIN_C[6], SIN_C[5], OP.mult, OP.add)
            for k in range(4, -1, -1):
                q = t16("psq" + ("a" if k % 2 else "b"))
                tt(q, ps, v2, OP.mult)
                psn = t16("ps" + ("a" if k % 2 else "b"))
                ts(psn, q, SIN_C[k], None, OP.add)
                ps = psn
            SIN = sc.tile([128, 4, G], F32, name="SIN", tag="SIN")
            SINF = SIN[:, :, :].rearrange("p a g -> p (a g)")
            tt(SINF, ps, vv, OP.mult)
            # cos(a): even poly (coeffs of -cos(v))
            pc = t16("pca")
            ts(pc, v2, COS_C[7], COS_C[6], OP.mult, OP.add)
            for k in range(5, -1, -1):
                qc = t16("pcq" + ("a" if k % 2 else "b"))
                tt(qc, pc, v2, OP.mult)
                pcn = t16("pc" + ("a" if k % 2 else "b"))
                ts(pcn, qc, COS_C[k], None, OP.add)
                pc = pcn
            COS = sc.tile([128, 4, G], F32, name="COS", tag="COS")
            COSF = COS[:, :, :].rearrange("p a g -> p (a g)")
            nc.vector.tensor_copy(out=COSF, in_=pc)

            NSIN = sc.tile([128, 4, G], F32, name="NSIN", tag="NSIN")
            nc.vector.tensor_scalar(
                out=NSIN[:, :, :].rearrange("p a g -> p (a g)"),
                in0=SINF, scalar1=-1.0, scalar2=None, op0=OP.mult,
            )

            # d_a = sin*x0 - cos*y0  -> DA[128, 4, G]
            DA = sc.tile([128, 4, G], F32, name="DA", tag="DA")
            sx = t4("sx"); cy = t4("cy")
            for i in range(4):
                tt(sx, SIN[:, i, :], x0, OP.mult)
                tt(cy, COS[:, i, :], y0, OP.mult)
                tt(DA[:, i, :], sx, cy, OP.subtract)

            # ---------------- accumulator tiles ----------------
            # R_act: S_c at cols c*4+g (0..11), H1 at 48+g, H2 at 52+g
            # R_dve: M1_c at c*4+g (0..11), M2_c at 16 + c*4+g (16..27)
            R_act = const.tile([128, 64], F32)
            R_dve = const.tile([128, 64], F32)

            # ---------------- main loop over 4 groups ----------------
            for g in range(GROUPS):
                patch = patchp.tile([128, R, 63], F32, name="patch", tag="patch")
                for dh in range(2):
                    row0 = 4 * g + 2 * dh
                    nc.sync.dma_start(
                        out=patch[dh * 64:(dh + 1) * 64, :, :],
                        in_=dram_ap(
                            img_ext, row0 * W * 3,
                            [[STRIDE * 3, 64], [W * 3, R], [1, 63]],
                        ),
                    )

                def scl(tile_, i):
                    return tile_[:, i, g:g + 1]

                # lines for a1..a4
                lines = []
                for i in range(4):
                    u = work.tile([128, NPIX], F32, name=f"lx{i}", tag=f"lx{i}")
                    nc.scalar.activation(
                        out=u, in_=X, func=ACT.Identity,
                        scale=scl(NSIN, i), bias=scl(DA, i),
                    )
                    w = work.tile([128, NPIX], F32, name=f"ly{i}", tag=f"ly{i}")
                    nc.vector.tensor_scalar(
                        out=w, in0=Y, scalar1=scl(COS, i), scalar2=None, op0=OP.mult
                    )
                    ln = work.tile([128, NPIX], F32, name=f"ln{i}", tag=f"ln{i}")
                    tt(ln, u, w, OP.add)
                    lines.append(ln)

                def dist(la, lb, sg, nsg, gt, tag):
                    ua = work.tile([128, NPIX], F32, name=tag + "u", tag=tag + "u")
                    nc.vector.tensor_scalar(out=ua, in0=la, scalar1=sg[:, g:g + 1], scalar2=None, op0=OP.mult)
                    ub = work.tile([128, NPIX], F32, name=tag + "v", tag=tag + "v")
                    nc.vector.tensor_scalar(out=ub, in0=lb, scalar1=nsg[:, g:g + 1], scalar2=None, op0=OP.mult)
                    mn = work.tile([128, NPIX], F32, name=tag + "m", tag=tag + "m")
                    tt(mn, ua, ub, OP.min)
                    d = work.tile([128, NPIX], F32, name=tag + "d", tag=tag + "d")
                    nc.vector.tensor_scalar(
                        out=d, in0=mn, scalar1=sg[:, g:g + 1], scalar2=gt[:, g:g + 1],
                        op0=OP.mult, op1=OP.add,
                    )
                    return d

                d13 = dist(lines[0], lines[2], sgn13, nsgn13, gt13, "d13")
                d42 = dist(lines[3], lines[1], sgn42, nsgn42, gt42, "d42")

                t0 = work.tile([128, NPIX], F32, name="t0", tag="t0")
                nc.scalar.activation(out=t0, in_=d13, func=ACT.Arctan, scale=1.0 / ETA)
                t1 = work.tile([128, NPIX], F32, name="t1", tag="t1")
                nc.scalar.activation(out=t1, in_=d42, func=ACT.Arctan, scale=1.0 / ETA)

                h0 = work.tile([128, NPIX], F32, name="h0", tag="h0")
                nc.vector.tensor_scalar(out=h0, in0=t0, scalar1=1.0 / PI, scalar2=0.5, op0=OP.mult, op1=OP.add)
                h1 = work.tile([128, NPIX], F32, name="h1", tag="h1")
                nc.vector.tensor_scalar(out=h1, in0=t1, scalar1=1.0 / PI, scalar2=0.5, op0=OP.mult, op1=OP.add)
                p01 = work.tile([128, NPIX], F32, name="p01", tag="p01")
                tt(p01, h0, h1, OP.mult)

                # H sums on ACT
                sa = scrp.tile([128, NPIX], F32, name="sa", tag="sa")
                nc.scalar.activation(out=sa, in_=h0, func=ACT.Copy, accum_out=R_act[:, 48 + g:49 + g])
                sa2 = scrp.tile([128, NPIX], F32, name="sa", tag="sa")
                nc.scalar.activation(out=sa2, in_=p01, func=ACT.Copy, accum_out=R_act[:, 52 + g:53 + g])

                for c in range(3):
                    Ic = patch[:, :, c::3]  # [128, 21, 21] strided
                    col = c * 4 + g
                    # S_c on ACT
                    so = scrp.tile([128, R, R], F32, name="so", tag="so")
                    nc.scalar.activation(out=so, in_=Ic, func=ACT.Copy, accum_out=R_act[:, col:col + 1])
                    # M1_c, M2_c on DVE (fused mult+accum)
                    m1o = scrp.tile([128, R, R], F32, name="m1o", tag="m1o")
                    nc.vector.scalar_tensor_tensor(
                        out=m1o, in0=h0[:, :].rearrange("p (r s) -> p r s", r=R),
                        scalar=1.0, in1=Ic, op0=OP.mult, op1=OP.mult,
                        accum_out=R_dve[:, col:col + 1],
                    )
                    m2o = scrp.tile([128, R, R], F32, name="m2o", tag="m2o")
                    nc.vector.scalar_tensor_tensor(
                        out=m2o, in0=p01[:, :].rearrange("p (r s) -> p r s", r=R),
                        scalar=1.0, in1=Ic, op0=OP.mult, op1=OP.mult,
                        accum_out=R_dve[:, col + 16:col + 17],
                    )

            # ---------------- epilogue ----------------
            Sv = R_act[:, 0:12].rearrange("p (c g) -> p c g", c=3)
            M1v = R_dve[:, 0:12].rearrange("p (c g) -> p c g", c=3)
            M2v = R_dve[:, 16:28].rearrange("p (c g) -> p c g", c=3)
            H1v = R_act[:, 48:52]
            H2v = R_act[:, 52:56]

            C = const.tile([128, 3, 3, GROUPS], F32)  # (c, k, g)
            tt(C[:, :, 0, :], Sv, M1v, OP.subtract)
            tt(C[:, :, 1, :], M1v, M2v, OP.subtract)
            nc.vector.tensor_copy(out=C[:, :, 2, :], in_=M2v)

            Wt = const.tile([128, 3, GROUPS], F32)  # (k, g)
            nc.vector.tensor_scalar(out=Wt[:, 0, :], in0=H1v, scalar1=-1.0, scalar2=float(NPIX), op0=OP.mult, op1=OP.add)
            tt(Wt[:, 1, :], H1v, H2v, OP.subtract)
            nc.vector.tensor_copy(out=Wt[:, 2, :], in_=H2v)

            WF = Wt[:, :, :].rearrange("p k g -> p (k g)")
            W2 = const.tile([128, 3 * GROUPS], F32)
            nc.vector.tensor_scalar(out=W2, in0=WF, scalar1=1e-10, scalar2=None, op0=OP.add)
            VW = const.tile([128, 3, GROUPS], F32)
            nc.vector.reciprocal(out=VW[:, :, :].rearrange("p k g -> p (k g)"), in_=W2)

            C2 = const.tile([128, 3, 3, GROUPS], F32)
            for c in range(3):
                tt(C2[:, c, :, :], C[:, c, :, :], VW[:, :, :], OP.mult)

            pt = psum.tile([36, 128], F32)
            nc.tensor.transpose(
                pt[:, :], C2[:, :, :, :].rearrange("p a b c -> p (a b c)"), ident[:, :]
            )
            T = const.tile([36, 128], F32)
            nc.scalar.copy(out=T, in_=pt)
            nc.sync.dma_start(
                out=out_ext[:, :].rearrange("a (b c) -> a b c", b=GROUPS),
                in_=T,
            )

    nc.finalize()
    return nc


_NC_CACHE = None


def _get_nc():
    global _NC_CACHE
    if _NC_CACHE is None:
        _NC_CACHE = build_nc()
    return _NC_CACHE


def make_in_maps(ests, noisy_image):
    """Build the 8 per-core input dicts from full inputs."""
    img = np.ascontiguousarray(np.asarray(noisy_image, dtype=np.float32)[0])  # (147,147,3)
    ests = np.asarray(ests, dtype=np.float32).reshape(HP * WP, 5)
    grid = np.linspace(-1.0, 1.0, R, dtype=np.float32)
    xg = np.tile(grid, R)                      # x[r,s] = grid[s]
    yg = np.repeat(grid, R)                    # y[r,s] = grid[r]
    ident = np.eye(128, dtype=np.float32)
    imgp = img  # last core reads rows 112..147 == H, exactly in bounds
    in_maps = []
    for m in range(NCORES):
        in_maps.append({
            "img": np.ascontiguousarray(imgp[16 * m:16 * m + ROWS_PER_CORE]).reshape(-1),
            "ests": np.ascontiguousarray(ests[m * 512:(m + 1) * 512]).reshape(-1),
            "xg": xg, "yg": yg, "ident": ident,
        })
    return in_maps


def assemble(results):
    out = np.empty((1, 3, 3, HP, WP), np.float32)
    for m in range(NCORES):
        out[0, :, :, 8 * m:8 * m + 8, :] = results[m]["out"].reshape(3, 3, 8, WP)
    return out


def kernel(ests, noisy_image, gt_image=None, alpha=None, **_):
    nc = _get_nc()
    in_maps = make_in_maps(ests, noisy_image)
    res = run_bass_kernel_spmd(nc, in_maps, core_ids=list(range(NCORES)))
    return assemble(res.results)
